# revision 1
# baseline (speedup 1.0000x reference)
"""ConvLSTM3D encoder kernel for 8 trn2 NeuronCores.

Sharding: core c in [0,8) handles batch b = c//4, z-slab k = c%4 (8 output
planes z in [8k, 8k+8)).  The sequential T=10 loop runs on-device; per-step
halo exchange (1 plane each side of the slab) goes through an AllGather over
the 4 cores of each batch group.

Conv mapping: gates = Wx (x) x_t (stride 2) + Wh (x) h + b is computed as a
single K=128 matmul accumulation stream per 512-voxel output chunk:
  partitions  0..95  : three z-shifted copies of h (dz = 0,1,2)
  partitions 96..122 : host-precomputed im2col taps of x_t (27 taps)
  partition  123     : ones (bias row)
For each (dy,dx) in 3x3, one matmul with an AP offset of (dy,dx) into the
padded (34x34) plane layout contracts channels x dz at once; the x-conv and
bias blocks ride along in the delta=(0,0) matmul only (their lhsT rows are
zero in the other eight).
"""

import os
import sys
from contextlib import ExitStack

import numpy as np

for _p in ("/opt/trn_rl_repo", "/root/.axon_site/_ro/trn_rl_repo"):
    if os.path.isdir(_p) and _p not in sys.path:
        sys.path.insert(0, _p)

import concourse.bass as bass
import concourse.bacc as bacc
import concourse.mybir as mybir
from concourse import tile
from concourse.bass_utils import run_bass_kernel_spmd

F32 = mybir.dt.float32
I32 = mybir.dt.int32
MM_DT = mybir.dt.float32r  # matmul operand dtype (1 cycle/row at N>=256)

T = 10
CH = 32          # hidden channels
NG = 128         # gate rows (4 gates x 32 ch)
SLAB = 8         # output planes per core
PLW = 34         # padded plane width
PL = PLW * PLW   # padded plane elements (1156)
HS_FREE = SLAB * PL  # h-stack free size per partition (9248)
DELTAS = [(dy, dx) for dy in range(3) for dx in range(3)]
# plane processing order: interior first, halo-dependent planes (0, 7) last
PI = [3, 4, 5, 2, 6, 1, 0, 7]
RG = [[0, 1, 2, 3, 4, 5, 6, 7]]

_prog_cache = {}


def _build_program(nsteps=T, halo=True):
    key = (nsteps, halo)
    if key in _prog_cache:
        return _prog_cache[key]

    nc = bacc.Bacc(num_devices=8)

    xim_d = nc.dram_tensor("xim", [T, 28, HS_FREE], MM_DT, kind="ExternalInput")
    whl_d = nc.dram_tensor("whl", [9, 128, 128], MM_DT, kind="ExternalInput")
    hoff_d = nc.dram_tensor("hoff", [1, 2], I32, kind="ExternalInput")
    zeros_d = nc.dram_tensor("zeros", [128, HS_FREE], MM_DT, kind="ExternalInput")
    hout_d = nc.dram_tensor("hout", [CH, SLAB, 32, 32], F32, kind="ExternalOutput")
    agin = nc.dram_tensor("agin", [3, CH, 1024], F32)
    agout = nc.dram_tensor("agout", [24, CH, 1024], F32, addr_space="Shared")

    with ExitStack() as ctx:
        tc = ctx.enter_context(tile.TileContext(nc))
        pers = ctx.enter_context(tc.tile_pool(name="pers", bufs=1))
        psum = ctx.enter_context(tc.tile_pool(name="psum", bufs=2, space="PSUM"))
        work = ctx.enter_context(tc.tile_pool(name="work", bufs=2))

        hstack = [
            pers.tile([128, HS_FREE], MM_DT, tag="hstackA", name="hstackA"),
            pers.tile([128, HS_FREE], MM_DT, tag="hstackB", name="hstackB"),
        ]
        wh_sb = pers.tile([128, 9 * 128], MM_DT, tag="wh")
        gates = pers.tile([128, 16 * 512], F32, tag="gates")
        c_state = pers.tile([128, 4 * 512], F32, tag="cstate")
        zscr = pers.tile([32, 1024], F32, tag="zscr")

        # ---- init ----
        nc.sync.dma_start(out=hstack[0][:, :], in_=zeros_d[:, :])
        nc.sync.dma_start(out=hstack[1][:, :], in_=zeros_d[:, :])
        nc.vector.memset(c_state[:, :], 0.0)
        nc.vector.memset(zscr[:, :], 0.0)
        nc.sync.dma_start(out=agin[2], in_=zscr[:, :])
        for _d in range(9):
            nc.sync.dma_start(out=wh_sb[:, 128 * _d:128 * (_d + 1)],
                              in_=whl_d[_d])
        nc.sync.dma_start(out=hstack[0][96:124, :], in_=xim_d[0])

        r_lo = nc.alloc_register(mybir.EngineType.Pool, "r_lo")
        r_hi = nc.alloc_register(mybir.EngineType.Pool, "r_hi")
        nc.reg_load(r_lo, hoff_d[0:1, 0:1])
        nc.reg_load(r_hi, hoff_d[0:1, 1:2])
        rv_lo = nc.snap(r_lo, min_val=0, max_val=23)
        rv_hi = nc.snap(r_hi, min_val=0, max_val=23)

        hsv = [h[:, :].rearrange("p (z y x) -> p z y x", z=SLAB, y=PLW, x=PLW)
               for h in hstack]

        # deferred tail-of-slice state (software pipelining of the emission)
        pending = []  # (t, l, c_sl, o_t, tanhc_t, h_t, nxtv)
        # round-robin engine assignment for the h-stack broadcast copies
        bcast_engines = [nc.gpsimd, nc.vector, nc.scalar]

        def emit_tail(t, l, c_sl, o_t, tanhc_t, h_t, nxtv):
            nc.scalar.activation(tanhc_t[:, :], c_sl, mybir.ActivationFunctionType.Tanh)
            nc.vector.tensor_mul(h_t[:, :], o_t[:, :], tanhc_t[:, :])
            last = t == nsteps - 1
            eng_i = 0
            for q in range(4):  # four planes in this slice
                pl = PI[4 * l + q]
                src = h_t[32 * q:32 * q + 32, :]
                src3 = src.rearrange("p (y x) -> p y x", y=32, x=32)
                if last:
                    nc.sync.dma_start(out=hout_d[:, pl, :, :], in_=src3.bitcast(F32))
                    continue
                for g in range(3):
                    pos = pl + 1 - g
                    if 0 <= pos <= 7:
                        eng = bcast_engines[eng_i % 3]
                        eng_i += 1
                        if eng is nc.scalar:
                            eng.copy(nxtv[32 * g:32 * g + 32, pos, 1:33, 1:33], src3)
                        else:
                            eng.tensor_copy(nxtv[32 * g:32 * g + 32, pos, 1:33, 1:33],
                                            src3)
                if pl == 0:
                    nc.sync.dma_start(out=agin[0], in_=src.bitcast(F32))
                elif pl == 7:
                    nc.sync.dma_start(out=agin[1], in_=src.bitcast(F32))

        T_ = nsteps
        for t in range(T_):
            cur, nxt = hstack[t % 2], hstack[(t + 1) % 2]
            curv, nxtv = hsv[t % 2], hsv[(t + 1) % 2]
            if t + 1 < T_:
                nc.sync.dma_start(out=nxt[96:124, :], in_=xim_d[t + 1])

            for l in range(2):  # two 4-plane slices per step
                gt = [work.tile([128, 1024], F32, tag=f"gate{G}", name=f"gate{G}")
                      for G in range(4)]
                for h in range(2):  # two psum tiles per slice
                    ps = psum.tile([128, 2048], F32, tag="ps", name="ps")
                    # delta-outer loop: 4 consecutive matmuls share one lhsT
                    if t == 0:
                        for cq in range(4):
                            o = 8 * l + 4 * h + cq
                            pl, hf = PI[o // 2], o % 2
                            r0 = 16 * hf
                            rhs = curv[:, pl, r0:r0 + 16, 0:32]
                            nc.tensor.matmul(ps[:, 512 * cq:512 * (cq + 1)],
                                             lhsT=wh_sb[:, 0:128],
                                             rhs=rhs, start=True, stop=True)
                    else:
                        for di, (dy, dx) in enumerate(DELTAS):
                            for cq in range(4):
                                o = 8 * l + 4 * h + cq
                                pl, hf = PI[o // 2], o % 2
                                r0 = 16 * hf
                                rhs = curv[:, pl, r0 + dy:r0 + dy + 16, dx:dx + 32]
                                nc.tensor.matmul(
                                    ps[:, 512 * cq:512 * (cq + 1)],
                                    lhsT=wh_sb[:, 128 * di:128 * (di + 1)],
                                    rhs=rhs,
                                    start=(di == 0), stop=(di == 8))
                    span = slice((8 * l + 4 * h) * 512, (8 * l + 4 * h + 4) * 512)
                    nc.scalar.activation(gates[0:96, span], ps[0:96, :],
                                         mybir.ActivationFunctionType.Sigmoid)
                    nc.scalar.activation(gates[96:128, span], ps[96:128, :],
                                         mybir.ActivationFunctionType.Tanh)
                    for G in range(4):
                        for q in (2 * h, 2 * h + 1):
                            nc.sync.dma_start(
                                out=gt[G][32 * q:32 * q + 32, :],
                                in_=gates[32 * G:32 * G + 32,
                                          (8 * l + 2 * q) * 512:
                                          (8 * l + 2 * q + 2) * 512])

                i_t, f_t, o_t, g_t = gt
                prod = work.tile([128, 1024], F32, tag="prod")
                tmp = work.tile([128, 1024], F32, tag="tmp")
                c_sl = c_state[:, 1024 * l:1024 * (l + 1)]
                nc.vector.tensor_mul(prod[:, :], i_t[:, :], g_t[:, :])
                nc.vector.tensor_mul(tmp[:, :], f_t[:, :], c_sl)
                nc.vector.tensor_add(c_sl, prod[:, :], tmp[:, :])

                tanhc = work.tile([128, 1024], F32, tag="tanhc")
                h_t = work.tile([128, 1024], MM_DT, tag="ht")
                if pending:
                    emit_tail(*pending.pop())
                pending.append((t, l, c_sl, o_t, tanhc, h_t, nxtv))

            # flush the last slice of this step before the collective
            if pending:
                emit_tail(*pending.pop())

            if t + 1 < T_ and halo:
                nc.gpsimd.collective_compute(
                    "AllGather", mybir.AluOpType.bypass, replica_groups=RG,
                    ins=[agin[:, :, :]], outs=[agout[:, :, :]])
                halo_lo = agout[bass.ds(rv_lo, 1)].squeeze(0).rearrange(
                    "c (y x) -> c y x", y=32, x=32)
                halo_hi = agout[bass.ds(rv_hi, 1)].squeeze(0).rearrange(
                    "c (y x) -> c y x", y=32, x=32)
                nc.gpsimd.dma_start(out=nxtv[0:32, 0, 1:33, 1:33],
                                    in_=halo_lo.bitcast(MM_DT))
                nc.gpsimd.dma_start(out=nxtv[64:96, 7, 1:33, 1:33],
                                    in_=halo_hi.bitcast(MM_DT))

    nc.finalize()
    _prog_cache[key] = nc
    return nc


def _host_inputs(input_batch, Wx, Wh, b):
    input_batch = np.asarray(input_batch, dtype=np.float32)
    Wx = np.asarray(Wx, dtype=np.float32)
    Wh = np.asarray(Wh, dtype=np.float32)
    b = np.asarray(b, dtype=np.float32)

    xp = np.zeros((2, T, 66, 66, 66), np.float32)
    xp[:, :, 1:65, 1:65, 1:65] = input_batch[:, :, 0]

    whl = np.zeros((9, 128, 128), np.float32)
    for di, (dy, dx) in enumerate(DELTAS):
        for g in range(3):
            whl[di, 32 * g:32 * g + 32, :] = Wh[:, :, g, dy, dx].T
    whl[0, 96:123, :] = Wx[:, 0].reshape(128, 27).T
    whl[0, 123, :] = b

    in_maps = []
    for c in range(8):
        bidx, k = divmod(c, 4)
        z0 = 8 * k
        xim = np.zeros((T, 28, SLAB, PLW, PLW), np.float32)
        for tz in range(3):
            for ty in range(3):
                for tx in range(3):
                    tap = tz * 9 + ty * 3 + tx
                    xim[:, tap, :, 0:32, 0:32] = xp[
                        bidx, :, 2 * z0 + tz:2 * z0 + tz + 16:2,
                        ty:ty + 64:2, tx:tx + 64:2]
        xim[:, 27, :, 0:32, 0:32] = 1.0
        lo_slot = c * 3 + 2 if k == 0 else (c - 1) * 3 + 1
        hi_slot = c * 3 + 2 if k == 3 else (c + 1) * 3 + 0
        in_maps.append({
            "xim": xim.reshape(T, 28, HS_FREE),
            "whl": whl,
            "hoff": np.array([[lo_slot, hi_slot]], np.int32),
            "zeros": np.zeros((128, HS_FREE), np.float32),
        })
    return in_maps


def run_cores(in_maps, nsteps=T, halo=True, **kwargs):
    nc = _build_program(nsteps, halo)
    return run_bass_kernel_spmd(nc, in_maps, list(range(8)), **kwargs)


def kernel(input_batch, Wx, Wh, b):
    in_maps = _host_inputs(input_batch, Wx, Wh, b)
    res = run_cores(in_maps)
    out = np.zeros((2, CH, 32, 32, 32), np.float32)
    for c in range(8):
        bidx, k = divmod(c, 4)
        out[bidx, :, 8 * k:8 * k + 8] = res.results[c]["hout"]
    return out



# revision 7
# speedup vs baseline: 1.1075x; 1.1075x over previous
"""ConvLSTM3D encoder kernel for 8 trn2 NeuronCores (v2, bf16).

Sharding: core c in [0,8) handles batch b = c//4, z-slab k = c%4 (8 output
planes z in [8k, 8k+8)).  The sequential T=10 loop runs on-device; per-step
halo exchange (1 plane each side of the slab) is an AllGather over the 4
cores of each batch group (bf16 payload, double-buffered DRAM).

Conv mapping: gates = Wx (x) x_t (stride 2) + Wh (x) h + b is one K=128
bf16 matmul accumulation stream per output plane (N=1024 = full 32x32):
  partitions  0..95  : three z-shifted copies of h (dz = 0,1,2)
  partitions 96..122 : host-precomputed im2col taps of x_t (27 taps)
  partition  123     : ones (bias row, memset once)
For each (dy,dx) in 3x3, one matmul with an AP offset of (dy,dx) into the
padded (34x34) plane layout contracts channels x dz at once; the x-conv and
bias ride in the delta=(0,0) matmul only (their lhsT rows are zero in the
other eight).

Elementwise LSTM math runs on [32, span] slices straight out of the gates
tile (i/f/o/g live on partition quadrants 0:32/32:64/64:96/96:128 - the DVE
crossbar allows different quadrant bases per operand at nch=32), cell state
is fp16 for the 2-byte DVE fast mode.  h is written once (strided, bf16)
into the dz=1 partition group of the next h-stack; the dz=0/2 groups are
produced by two large contiguous SBUF->SBUF DMAs with a +-1 plane offset.
Boundary planes (0,7) are computed first each step so the halo collective
overlaps the interior-plane compute.
"""

import os
import sys
from contextlib import ExitStack

import numpy as np
import ml_dtypes

for _p in ("/opt/trn_rl_repo", "/root/.axon_site/_ro/trn_rl_repo"):
    if os.path.isdir(_p) and _p not in sys.path:
        sys.path.insert(0, _p)

import concourse.bass as bass
import concourse.bacc as bacc
import concourse.mybir as mybir
from concourse import tile
from concourse.bass_utils import run_bass_kernel_spmd

F32 = mybir.dt.float32
F16 = mybir.dt.float16
BF = mybir.dt.bfloat16
I32 = mybir.dt.int32

T = 10
CH = 32          # hidden channels
SLAB = 8         # output planes per core
PLW = 34         # padded plane width
PL = PLW * PLW   # padded plane elements (1156)
HS_FREE = SLAB * PL  # h-stack free size per partition (9248)
DELTAS = [(dy, dx) for dy in range(3) for dx in range(3)]
# plane processing order: boundary planes first (their h feeds the
# collective), interior after (overlaps the collective in flight)
PO = [0, 7, 1, 2, 3, 4, 5, 6]
SPAN = {p: i * 1024 for i, p in enumerate(PO)}
RG = [[0, 1, 2, 3, 4, 5, 6, 7]]
NPBF = ml_dtypes.bfloat16

_prog_cache = {}


def _build_program(nsteps=T):
    key = nsteps
    if key in _prog_cache:
        return _prog_cache[key]

    nc = bacc.Bacc(num_devices=8)

    xim_d = nc.dram_tensor("xim", [T, 27, HS_FREE], BF, kind="ExternalInput")
    whl_d = nc.dram_tensor("whl", [9, 128, 128], BF, kind="ExternalInput")
    hoff_d = nc.dram_tensor("hoff", [1, 2], I32, kind="ExternalInput")
    ones_d = nc.dram_tensor("ones", [1, HS_FREE], BF, kind="ExternalInput")
    hout_d = nc.dram_tensor("hout", [CH, SLAB, 32, 32], F32, kind="ExternalOutput")
    agin = [nc.dram_tensor(f"agin{i}", [3, CH, 1024], BF) for i in range(2)]
    agout = [nc.dram_tensor(f"agout{i}", [24, CH, 1024], BF, addr_space="Shared")
             for i in range(2)]

    with ExitStack() as ctx:
        tc = ctx.enter_context(tile.TileContext(nc))
        pers = ctx.enter_context(tc.tile_pool(name="pers", bufs=1))
        psum = ctx.enter_context(tc.tile_pool(name="psum", bufs=4, space="PSUM"))
        work = ctx.enter_context(tc.tile_pool(name="work", bufs=2))

        hstack = [
            pers.tile([128, HS_FREE], BF, tag="hstackA", name="hstackA"),
            pers.tile([128, HS_FREE], BF, tag="hstackB", name="hstackB"),
        ]
        wh_sb = pers.tile([128, 9 * 128], BF, tag="wh")
        # elementwise operand placement: every two-input DVE op needs both
        # inputs on the same base partition, so: g at base 0 (own tile),
        # c/prod/tmp at base 32, tanh(c) at base 64 (next to o).
        c_state = pers.tile([64, 8 * 1024], F16, tag="cstate")
        prod = pers.tile([64, 8 * 1024], F16, tag="prod")
        tmp = pers.tile([64, 8 * 1024], F16, tag="tmp")
        tanhc = pers.tile([96, 8 * 1024], F16, tag="tanhc")
        hfin = pers.tile([32, 8 * 1024], F32, tag="hfin")
        zscr = pers.tile([CH, 1024], BF, tag="zscr")

        # ---- init ----
        nc.vector.memset(hstack[0][:, :], 0.0)
        nc.gpsimd.memset(hstack[1][:, :], 0.0)
        nc.vector.memset(c_state[32:64, :], 0.0)
        nc.vector.memset(zscr[:, :], 0.0)
        for i in range(2):
            nc.sync.dma_start(out=hstack[i][123:124, :], in_=ones_d[:, :])
        for i in range(2):
            nc.sync.dma_start(out=agin[i][2], in_=zscr[:, :])
        for _d in range(9):
            nc.sync.dma_start(out=wh_sb[:, 128 * _d:128 * (_d + 1)],
                              in_=whl_d[_d])
        nc.sync.dma_start(out=hstack[0][96:123, :], in_=xim_d[0])

        r_lo = nc.alloc_register(mybir.EngineType.Pool, "r_lo")
        r_hi = nc.alloc_register(mybir.EngineType.Pool, "r_hi")
        nc.reg_load(r_lo, hoff_d[0:1, 0:1])
        nc.reg_load(r_hi, hoff_d[0:1, 1:2])
        rv_lo = nc.snap(r_lo, min_val=0, max_val=23)
        rv_hi = nc.snap(r_hi, min_val=0, max_val=23)

        hsv = [h[:, :].rearrange("p (z y x) -> p z y x", z=SLAB, y=PLW, x=PLW)
               for h in hstack]

        def emit_plane(t, curv, gates, p):
            """9-delta matmul accumulation + gate activations for plane p."""
            ps = psum.tile([128, 1024], F32, tag="ps", name="ps")
            if t == 0:
                for h in range(2):
                    nc.tensor.matmul(ps[:, 512 * h:512 * (h + 1)],
                                     lhsT=wh_sb[:, 0:128],
                                     rhs=curv[:, p, 16 * h:16 * h + 16, 0:32],
                                     start=True, stop=True)
            else:
                for di, (dy, dx) in enumerate(DELTAS):
                    for h in range(2):
                        nc.tensor.matmul(
                            ps[:, 512 * h:512 * (h + 1)],
                            lhsT=wh_sb[:, 128 * di:128 * (di + 1)],
                            rhs=curv[:, p, 16 * h + dy:16 * h + dy + 16,
                                     dx:dx + 32],
                            start=(di == 0), stop=(di == 8))
            s = SPAN[p]
            nc.scalar.activation(gates[0:96, s:s + 1024], ps[0:96, :],
                                 mybir.ActivationFunctionType.Sigmoid)
            nc.scalar.activation(g_t[0:32, s:s + 1024], ps[96:128, :],
                                 mybir.ActivationFunctionType.Tanh)

        def emit_group(t, gates, g_t, nxtv, planes, s0, s1):
            """LSTM elementwise update for gate span [s0:s1] (planes list)."""
            i_sl = gates[0:32, s0:s1]
            f_sl = gates[32:64, s0:s1]
            o_sl = gates[64:96, s0:s1]
            c_sl = c_state[32:64, s0:s1]
            nc.vector.tensor_mul(prod[32:64, s0:s1], i_sl, g_t[0:32, s0:s1])
            nc.vector.tensor_mul(tmp[32:64, s0:s1], f_sl, c_sl)
            nc.vector.tensor_add(c_sl, tmp[32:64, s0:s1], prod[32:64, s0:s1])
            nc.scalar.activation(tanhc[64:96, s0:s1], c_sl,
                                 mybir.ActivationFunctionType.Tanh)
            last = t == nsteps - 1
            for pl, a, b in planes:
                o_ap = o_sl[:, a - s0:b - s0].rearrange(
                    "p (z y x) -> p z y x", z=(b - a) // 1024, y=32, x=32)
                t_ap = tanhc[64:96, a:b].rearrange(
                    "p (z y x) -> p z y x", z=(b - a) // 1024, y=32, x=32)
                if last:
                    nc.vector.tensor_mul(
                        hfin[:, a:b].rearrange("p (z y x) -> p z y x",
                                               z=(b - a) // 1024, y=32, x=32),
                        o_ap, t_ap)
                else:
                    npl = (b - a) // 1024
                    nc.vector.tensor_mul(
                        nxtv[32:64, pl:pl + npl, 1:33, 1:33], o_ap, t_ap)

        T_ = nsteps
        for t in range(T_):
            cur, nxt = hstack[t % 2], hstack[(t + 1) % 2]
            curv, nxtv = hsv[t % 2], hsv[(t + 1) % 2]
            last = t == T_ - 1
            gates = work.tile([96, 8 * 1024], BF, tag="gates", name="gates")
            g_t = work.tile([32, 8 * 1024], BF, tag="g_t", name="g_t")
            if not last:
                nc.sync.dma_start(out=nxt[96:123, :], in_=xim_d[t + 1])

            # boundary planes first; their h feeds this step's collective
            emit_plane(t, curv, gates, 0)
            emit_plane(t, curv, gates, 7)
            emit_group(t, gates, g_t, nxtv, [(0, 0, 1024), (7, 1024, 2048)],
                       0, 2048)
            if not last:
                ag_i, ag_o = agin[t % 2], agout[t % 2]
                nc.sync.dma_start(
                    out=ag_i[0].rearrange("c (y x) -> c y x", y=32, x=32),
                    in_=nxtv[32:64, 0, 1:33, 1:33])
                nc.sync.dma_start(
                    out=ag_i[1].rearrange("c (y x) -> c y x", y=32, x=32),
                    in_=nxtv[32:64, 7, 1:33, 1:33])
                nc.gpsimd.collective_compute(
                    "AllGather", mybir.AluOpType.bypass, replica_groups=RG,
                    ins=[ag_i[:, :, :]], outs=[ag_o[:, :, :]])

            # interior planes (overlap the collective)
            for p in (1, 2, 3, 4, 5, 6):
                emit_plane(t, curv, gates, p)
            emit_group(t, gates, g_t, nxtv, [(1, 2048, 8192)], 2048, 8192)

            if not last:
                halo_lo = ag_o[bass.ds(rv_lo, 1)].squeeze(0).rearrange(
                    "c (y x) -> c y x", y=32, x=32)
                halo_hi = ag_o[bass.ds(rv_hi, 1)].squeeze(0).rearrange(
                    "c (y x) -> c y x", y=32, x=32)
                nc.gpsimd.dma_start(out=nxtv[0:32, 0, 1:33, 1:33], in_=halo_lo)
                nc.gpsimd.dma_start(out=nxtv[64:96, 7, 1:33, 1:33], in_=halo_hi)
                # dz=0 / dz=2 replicas: contiguous plane-shifted copies
                nc.sync.dma_start(out=nxtv[0:32, 1:8, :, :],
                                  in_=nxtv[32:64, 0:7, :, :])
                nc.sync.dma_start(out=nxtv[64:96, 0:7, :, :],
                                  in_=nxtv[32:64, 1:8, :, :])
            else:
                for pl in range(SLAB):
                    s = SPAN[pl]
                    nc.sync.dma_start(
                        out=hout_d[:, pl, :, :],
                        in_=hfin[:, s:s + 1024].rearrange(
                            "c (y x) -> c y x", y=32, x=32))

    nc.finalize()
    _prog_cache[key] = nc
    return nc


def _host_inputs(input_batch, Wx, Wh, b):
    input_batch = np.asarray(input_batch, dtype=np.float32)
    Wx = np.asarray(Wx, dtype=np.float32)
    Wh = np.asarray(Wh, dtype=np.float32)
    b = np.asarray(b, dtype=np.float32)

    xp = np.zeros((2, T, 66, 66, 66), np.float32)
    xp[:, :, 1:65, 1:65, 1:65] = input_batch[:, :, 0]

    whl = np.zeros((9, 128, 128), np.float32)
    for di, (dy, dx) in enumerate(DELTAS):
        for g in range(3):
            whl[di, 32 * g:32 * g + 32, :] = Wh[:, :, g, dy, dx].T
    whl[0, 96:123, :] = Wx[:, 0].reshape(128, 27).T
    whl[0, 123, :] = b
    whl = whl.astype(NPBF)

    in_maps = []
    for c in range(8):
        bidx, k = divmod(c, 4)
        z0 = 8 * k
        xim = np.zeros((T, 27, SLAB, PLW, PLW), np.float32)
        for tz in range(3):
            for ty in range(3):
                for tx in range(3):
                    tap = tz * 9 + ty * 3 + tx
                    xim[:, tap, :, 0:32, 0:32] = xp[
                        bidx, :, 2 * z0 + tz:2 * z0 + tz + 16:2,
                        ty:ty + 64:2, tx:tx + 64:2]
        lo_slot = 3 * c + 2 if k == 0 else 3 * (c - 1) + 1
        hi_slot = 3 * c + 2 if k == 3 else 3 * (c + 1)
        in_maps.append({
            "xim": xim.reshape(T, 27, HS_FREE).astype(NPBF),
            "whl": whl,
            "ones": np.ones((1, HS_FREE), NPBF),
            "hoff": np.array([[lo_slot, hi_slot]], np.int32),
        })
    return in_maps


def run_cores(in_maps, nsteps=T, **kwargs):
    nc = _build_program(nsteps)
    return run_bass_kernel_spmd(nc, in_maps, list(range(8)), **kwargs)


def kernel(input_batch, Wx, Wh, b):
    in_maps = _host_inputs(input_batch, Wx, Wh, b)
    res = run_cores(in_maps)
    out = np.zeros((2, CH, 32, 32, 32), np.float32)
    for c in range(8):
        bidx, k = divmod(c, 4)
        out[bidx, :, 8 * k:8 * k + 8] = res.results[c]["hout"]
    return out


# revision 8
# speedup vs baseline: 1.2473x; 1.1262x over previous
"""ConvLSTM3D encoder kernel for 8 trn2 NeuronCores (v2, bf16).

Sharding: core c in [0,8) handles batch b = c//4, z-slab k = c%4 (8 output
planes z in [8k, 8k+8)).  The sequential T=10 loop runs on-device; per-step
halo exchange (1 plane each side of the slab) is an AllGather over the 4
cores of each batch group (bf16 payload, double-buffered DRAM).

Conv mapping: gates = Wx (x) x_t (stride 2) + Wh (x) h + b is one K=128
bf16 matmul accumulation stream per output plane (N=1024 = full 32x32):
  partitions  0..95  : three z-shifted copies of h (dz = 0,1,2)
  partitions 96..122 : host-precomputed im2col taps of x_t (27 taps)
  partition  123     : ones (bias row, memset once)
For each (dy,dx) in 3x3, one matmul with an AP offset of (dy,dx) into the
padded (34x34) plane layout contracts channels x dz at once; the x-conv and
bias ride in the delta=(0,0) matmul only (their lhsT rows are zero in the
other eight).

Elementwise LSTM math runs on [32, span] slices straight out of the gates
tile (i/f/o/g live on partition quadrants 0:32/32:64/64:96/96:128 - the DVE
crossbar allows different quadrant bases per operand at nch=32), cell state
is fp16 for the 2-byte DVE fast mode.  h is written once (strided, bf16)
into the dz=1 partition group of the next h-stack; the dz=0/2 groups are
produced by two large contiguous SBUF->SBUF DMAs with a +-1 plane offset.
Boundary planes (0,7) are computed first each step so the halo collective
overlaps the interior-plane compute.
"""

import os
import sys
from contextlib import ExitStack

import numpy as np
import ml_dtypes

for _p in ("/opt/trn_rl_repo", "/root/.axon_site/_ro/trn_rl_repo"):
    if os.path.isdir(_p) and _p not in sys.path:
        sys.path.insert(0, _p)

import concourse.bass as bass
import concourse.bacc as bacc
import concourse.mybir as mybir
from concourse import tile
from concourse.bass_utils import run_bass_kernel_spmd

F32 = mybir.dt.float32
F16 = mybir.dt.float16
BF = mybir.dt.bfloat16
I32 = mybir.dt.int32

T = 10
CH = 32          # hidden channels
SLAB = 8         # output planes per core
PLW = 34         # padded plane width
PL = PLW * PLW   # padded plane elements (1156)
HS_FREE = SLAB * PL  # h-stack free size per partition (9248)
DELTAS = [(dy, dx) for dy in range(3) for dx in range(3)]
# plane processing order: boundary planes first (their h feeds the
# collective), interior after (overlaps the collective in flight)
PO = [0, 7, 1, 2, 3, 4, 5, 6]
SPAN = {p: i * 1024 for i, p in enumerate(PO)}
RG = [[0, 1, 2, 3, 4, 5, 6, 7]]
NPBF = ml_dtypes.bfloat16

_prog_cache = {}


def _build_program(nsteps=T):
    key = nsteps
    if key in _prog_cache:
        return _prog_cache[key]

    nc = bacc.Bacc(num_devices=8)

    xim_d = nc.dram_tensor("xim", [T, 27, HS_FREE], BF, kind="ExternalInput")
    whl_d = nc.dram_tensor("whl", [9, 128, 128], BF, kind="ExternalInput")
    hoff_d = nc.dram_tensor("hoff", [1, 2], I32, kind="ExternalInput")
    ones_d = nc.dram_tensor("ones", [1, HS_FREE], BF, kind="ExternalInput")
    hout_d = nc.dram_tensor("hout", [CH, SLAB, 32, 32], F32, kind="ExternalOutput")
    agin = [nc.dram_tensor(f"agin{i}", [3, CH, 1024], BF) for i in range(2)]
    agout = [nc.dram_tensor(f"agout{i}", [24, CH, 1024], BF, addr_space="Shared")
             for i in range(2)]

    with ExitStack() as ctx:
        tc = ctx.enter_context(tile.TileContext(nc))
        pers = ctx.enter_context(tc.tile_pool(name="pers", bufs=1))
        psum = ctx.enter_context(tc.tile_pool(name="psum", bufs=4, space="PSUM"))
        work = ctx.enter_context(tc.tile_pool(name="work", bufs=2))

        hstack = [
            pers.tile([128, HS_FREE], BF, tag="hstackA", name="hstackA"),
            pers.tile([128, HS_FREE], BF, tag="hstackB", name="hstackB"),
        ]
        wh_sb = pers.tile([128, 9 * 128], BF, tag="wh")
        # elementwise operand placement: every two-input DVE op needs both
        # inputs on the same base partition, so: g at base 0 (own tile),
        # c/prod/tmp at base 32, tanh(c) at base 64 (next to o).
        c_state = pers.tile([64, 8 * 1024], F16, tag="cstate")
        prod = pers.tile([64, 8 * 1024], F16, tag="prod")
        tmp = pers.tile([64, 8 * 1024], F16, tag="tmp")
        tanhc = pers.tile([96, 8 * 1024], F16, tag="tanhc")
        hfin = pers.tile([32, 8 * 1024], F32, tag="hfin")
        zscr = pers.tile([CH, 1024], BF, tag="zscr")

        # ---- init ----
        nc.vector.memset(hstack[0][:, :], 0.0)
        nc.gpsimd.memset(hstack[1][:, :], 0.0)
        nc.vector.memset(c_state[32:64, :], 0.0)
        nc.vector.memset(zscr[:, :], 0.0)
        for i in range(2):
            nc.sync.dma_start(out=hstack[i][123:124, :], in_=ones_d[:, :])
        for i in range(2):
            nc.sync.dma_start(out=agin[i][2], in_=zscr[:, :])
        for _d in range(9):
            nc.sync.dma_start(out=wh_sb[:, 128 * _d:128 * (_d + 1)],
                              in_=whl_d[_d])
        nc.sync.dma_start(out=hstack[0][96:123, :], in_=xim_d[0])

        r_lo = nc.alloc_register(mybir.EngineType.Pool, "r_lo")
        r_hi = nc.alloc_register(mybir.EngineType.Pool, "r_hi")
        nc.reg_load(r_lo, hoff_d[0:1, 0:1])
        nc.reg_load(r_hi, hoff_d[0:1, 1:2])
        rv_lo = nc.snap(r_lo, min_val=0, max_val=23)
        rv_hi = nc.snap(r_hi, min_val=0, max_val=23)

        hsv = [h[:, :].rearrange("p (z y x) -> p z y x", z=SLAB, y=PLW, x=PLW)
               for h in hstack]

        def emit_plane(t, curv, gates, p):
            """9-delta matmul accumulation + gate activations for plane p."""
            ps = psum.tile([128, 1024], F32, tag="ps", name="ps")
            if t == 0:
                for h in range(2):
                    nc.tensor.matmul(ps[:, 512 * h:512 * (h + 1)],
                                     lhsT=wh_sb[:, 0:128],
                                     rhs=curv[:, p, 16 * h:16 * h + 16, 0:32],
                                     start=True, stop=True)
            else:
                for di, (dy, dx) in enumerate(DELTAS):
                    for h in range(2):
                        nc.tensor.matmul(
                            ps[:, 512 * h:512 * (h + 1)],
                            lhsT=wh_sb[:, 128 * di:128 * (di + 1)],
                            rhs=curv[:, p, 16 * h + dy:16 * h + dy + 16,
                                     dx:dx + 32],
                            start=(di == 0), stop=(di == 8))
            s = SPAN[p]
            nc.scalar.activation(gates[0:96, s:s + 1024], ps[0:96, :],
                                 mybir.ActivationFunctionType.Sigmoid)
            nc.scalar.activation(g_t[0:32, s:s + 1024], ps[96:128, :],
                                 mybir.ActivationFunctionType.Tanh)

        def emit_group(t, gates, g_t, nxtv, planes, s0, s1):
            """LSTM elementwise update for gate span [s0:s1] (planes list)."""
            i_sl = gates[0:32, s0:s1]
            f_sl = gates[32:64, s0:s1]
            o_sl = gates[64:96, s0:s1]
            c_sl = c_state[32:64, s0:s1]
            nc.vector.tensor_mul(prod[32:64, s0:s1], i_sl, g_t[0:32, s0:s1])
            nc.gpsimd.tensor_mul(tmp[32:64, s0:s1], f_sl, c_sl)
            nc.vector.tensor_add(c_sl, tmp[32:64, s0:s1], prod[32:64, s0:s1])
            nc.scalar.activation(tanhc[64:96, s0:s1], c_sl,
                                 mybir.ActivationFunctionType.Tanh)
            last = t == nsteps - 1
            for pl, a, b in planes:
                o_ap = o_sl[:, a - s0:b - s0].rearrange(
                    "p (z y x) -> p z y x", z=(b - a) // 1024, y=32, x=32)
                t_ap = tanhc[64:96, a:b].rearrange(
                    "p (z y x) -> p z y x", z=(b - a) // 1024, y=32, x=32)
                if last:
                    nc.vector.tensor_mul(
                        hfin[:, a:b].rearrange("p (z y x) -> p z y x",
                                               z=(b - a) // 1024, y=32, x=32),
                        o_ap, t_ap)
                else:
                    npl = (b - a) // 1024
                    nc.vector.tensor_mul(
                        nxtv[32:64, pl:pl + npl, 1:33, 1:33], o_ap, t_ap)

        T_ = nsteps
        for t in range(T_):
            cur, nxt = hstack[t % 2], hstack[(t + 1) % 2]
            curv, nxtv = hsv[t % 2], hsv[(t + 1) % 2]
            last = t == T_ - 1
            gates = work.tile([96, 8 * 1024], BF, tag="gates", name="gates")
            g_t = work.tile([32, 8 * 1024], BF, tag="g_t", name="g_t")
            if not last:
                nc.sync.dma_start(out=nxt[96:123, :], in_=xim_d[t + 1])

            # boundary planes first; their h feeds this step's collective
            emit_plane(t, curv, gates, 0)
            emit_plane(t, curv, gates, 7)
            emit_group(t, gates, g_t, nxtv, [(0, 0, 1024), (7, 1024, 2048)],
                       0, 2048)
            if not last:
                ag_i, ag_o = agin[t % 2], agout[t % 2]
                nc.sync.dma_start(
                    out=ag_i[0].rearrange("c (y x) -> c y x", y=32, x=32),
                    in_=nxtv[32:64, 0, 1:33, 1:33])
                nc.sync.dma_start(
                    out=ag_i[1].rearrange("c (y x) -> c y x", y=32, x=32),
                    in_=nxtv[32:64, 7, 1:33, 1:33])
                nc.gpsimd.collective_compute(
                    "AllGather", mybir.AluOpType.bypass, replica_groups=RG,
                    ins=[ag_i[:, :, :]], outs=[ag_o[:, :, :]])
                # g2 slot 6 <- h7 available right after the boundary group
                nc.sync.dma_start(out=nxtv[64:96, 6:7, :, :],
                                  in_=nxtv[32:64, 7:8, :, :])

            # interior planes in pairs (chunked elementwise + dz copies so
            # next-step matmuls wait only on per-plane producers)
            for p0 in (1, 3, 5):
                emit_plane(t, curv, gates, p0)
                emit_plane(t, curv, gates, p0 + 1)
                s0 = SPAN[p0]
                emit_group(t, gates, g_t, nxtv, [(p0, s0, s0 + 2048)],
                           s0, s0 + 2048)
                if not last:
                    # g0 slot p <- h[p-1]; g2 slot p <- h[p+1]
                    nc.sync.dma_start(out=nxtv[0:32, p0:p0 + 2, :, :],
                                      in_=nxtv[32:64, p0 - 1:p0 + 1, :, :])
                    nc.sync.dma_start(out=nxtv[64:96, p0 - 1:p0 + 1, :, :],
                                      in_=nxtv[32:64, p0:p0 + 2, :, :])

            if not last:
                # leftover replica slots: g0 slot 7 <- h6
                nc.sync.dma_start(out=nxtv[0:32, 7:8, :, :],
                                  in_=nxtv[32:64, 6:7, :, :])
                halo_lo = ag_o[bass.ds(rv_lo, 1)].squeeze(0).rearrange(
                    "c (y x) -> c y x", y=32, x=32)
                halo_hi = ag_o[bass.ds(rv_hi, 1)].squeeze(0).rearrange(
                    "c (y x) -> c y x", y=32, x=32)
                nc.gpsimd.dma_start(out=nxtv[0:32, 0, 1:33, 1:33], in_=halo_lo)
                nc.gpsimd.dma_start(out=nxtv[64:96, 7, 1:33, 1:33], in_=halo_hi)
            else:
                for pl in range(SLAB):
                    s = SPAN[pl]
                    nc.sync.dma_start(
                        out=hout_d[:, pl, :, :],
                        in_=hfin[:, s:s + 1024].rearrange(
                            "c (y x) -> c y x", y=32, x=32))

    nc.finalize()
    _prog_cache[key] = nc
    return nc


def _host_inputs(input_batch, Wx, Wh, b):
    input_batch = np.asarray(input_batch, dtype=np.float32)
    Wx = np.asarray(Wx, dtype=np.float32)
    Wh = np.asarray(Wh, dtype=np.float32)
    b = np.asarray(b, dtype=np.float32)

    xp = np.zeros((2, T, 66, 66, 66), np.float32)
    xp[:, :, 1:65, 1:65, 1:65] = input_batch[:, :, 0]

    whl = np.zeros((9, 128, 128), np.float32)
    for di, (dy, dx) in enumerate(DELTAS):
        for g in range(3):
            whl[di, 32 * g:32 * g + 32, :] = Wh[:, :, g, dy, dx].T
    whl[0, 96:123, :] = Wx[:, 0].reshape(128, 27).T
    whl[0, 123, :] = b
    whl = whl.astype(NPBF)

    in_maps = []
    for c in range(8):
        bidx, k = divmod(c, 4)
        z0 = 8 * k
        xim = np.zeros((T, 27, SLAB, PLW, PLW), np.float32)
        for tz in range(3):
            for ty in range(3):
                for tx in range(3):
                    tap = tz * 9 + ty * 3 + tx
                    xim[:, tap, :, 0:32, 0:32] = xp[
                        bidx, :, 2 * z0 + tz:2 * z0 + tz + 16:2,
                        ty:ty + 64:2, tx:tx + 64:2]
        lo_slot = 3 * c + 2 if k == 0 else 3 * (c - 1) + 1
        hi_slot = 3 * c + 2 if k == 3 else 3 * (c + 1)
        in_maps.append({
            "xim": xim.reshape(T, 27, HS_FREE).astype(NPBF),
            "whl": whl,
            "ones": np.ones((1, HS_FREE), NPBF),
            "hoff": np.array([[lo_slot, hi_slot]], np.int32),
        })
    return in_maps


def run_cores(in_maps, nsteps=T, **kwargs):
    nc = _build_program(nsteps)
    return run_bass_kernel_spmd(nc, in_maps, list(range(8)), **kwargs)


def kernel(input_batch, Wx, Wh, b):
    in_maps = _host_inputs(input_batch, Wx, Wh, b)
    res = run_cores(in_maps)
    out = np.zeros((2, CH, 32, 32, 32), np.float32)
    for c in range(8):
        bidx, k = divmod(c, 4)
        out[bidx, :, 8 * k:8 * k + 8] = res.results[c]["hout"]
    return out


# revision 9
# speedup vs baseline: 1.3289x; 1.0655x over previous
"""ConvLSTM3D encoder kernel for 8 trn2 NeuronCores (v2, bf16).

Sharding: core c in [0,8) handles batch b = c//4, z-slab k = c%4 (8 output
planes z in [8k, 8k+8)).  The sequential T=10 loop runs on-device; per-step
halo exchange (1 plane each side of the slab) is an AllGather over the 4
cores of each batch group (bf16 payload, double-buffered DRAM).

Conv mapping: gates = Wx (x) x_t (stride 2) + Wh (x) h + b is one K=128
bf16 matmul accumulation stream per output plane (N=1024 = full 32x32):
  partitions  0..95  : three z-shifted copies of h (dz = 0,1,2)
  partitions 96..122 : host-precomputed im2col taps of x_t (27 taps)
  partition  123     : ones (bias row, memset once)
For each (dy,dx) in 3x3, one matmul with an AP offset of (dy,dx) into the
padded (34x34) plane layout contracts channels x dz at once; the x-conv and
bias ride in the delta=(0,0) matmul only (their lhsT rows are zero in the
other eight).

Elementwise LSTM math runs on [32, span] slices straight out of the gates
tile (i/f/o/g live on partition quadrants 0:32/32:64/64:96/96:128 - the DVE
crossbar allows different quadrant bases per operand at nch=32), cell state
is fp16 for the 2-byte DVE fast mode.  h is written once (strided, bf16)
into the dz=1 partition group of the next h-stack; the dz=0/2 groups are
produced by two large contiguous SBUF->SBUF DMAs with a +-1 plane offset.
Boundary planes (0,7) are computed first each step so the halo collective
overlaps the interior-plane compute.
"""

import os
import sys
from contextlib import ExitStack

import numpy as np
import ml_dtypes

for _p in ("/opt/trn_rl_repo", "/root/.axon_site/_ro/trn_rl_repo"):
    if os.path.isdir(_p) and _p not in sys.path:
        sys.path.insert(0, _p)

import concourse.bass as bass
import concourse.bacc as bacc
import concourse.mybir as mybir
from concourse import tile
from concourse.bass_utils import run_bass_kernel_spmd

F32 = mybir.dt.float32
F16 = mybir.dt.float16
BF = mybir.dt.bfloat16
I32 = mybir.dt.int32

T = 10
CH = 32          # hidden channels
SLAB = 8         # output planes per core
PLW = 34         # padded plane width
PL = PLW * PLW   # padded plane elements (1156)
HS_FREE = SLAB * PL  # h-stack free size per partition (9248)
DELTAS = [(dy, dx) for dy in range(3) for dx in range(3)]
# plane processing order: boundary planes first (their h feeds the
# collective), interior after (overlaps the collective in flight)
PO = [0, 7, 1, 2, 3, 4, 5, 6]
SPAN = {p: i * 1024 for i, p in enumerate(PO)}
RG = [[0, 1, 2, 3], [4, 5, 6, 7]]
NPBF = ml_dtypes.bfloat16

_prog_cache = {}


def _build_program(nsteps=T):
    key = nsteps
    if key in _prog_cache:
        return _prog_cache[key]

    nc = bacc.Bacc(num_devices=8)

    xim_d = nc.dram_tensor("xim", [T, 27, HS_FREE], BF, kind="ExternalInput")
    whl_d = nc.dram_tensor("whl", [9, 128, 128], BF, kind="ExternalInput")
    hoff_d = nc.dram_tensor("hoff", [1, 2], I32, kind="ExternalInput")
    ones_d = nc.dram_tensor("ones", [1, HS_FREE], BF, kind="ExternalInput")
    hout_d = nc.dram_tensor("hout", [CH, SLAB, 32, 32], F32, kind="ExternalOutput")
    agin = [nc.dram_tensor(f"agin{i}", [3, CH, 1024], BF) for i in range(2)]
    agout = [nc.dram_tensor(f"agout{i}", [12, CH, 1024], BF)
             for i in range(2)]

    with ExitStack() as ctx:
        tc = ctx.enter_context(tile.TileContext(nc))
        pers = ctx.enter_context(tc.tile_pool(name="pers", bufs=1))
        psum = ctx.enter_context(tc.tile_pool(name="psum", bufs=4, space="PSUM"))
        work = ctx.enter_context(tc.tile_pool(name="work", bufs=2))

        hstack = [
            pers.tile([128, HS_FREE], BF, tag="hstackA", name="hstackA"),
            pers.tile([128, HS_FREE], BF, tag="hstackB", name="hstackB"),
        ]
        wh_sb = pers.tile([128, 9 * 128], BF, tag="wh")
        # elementwise operand placement: every two-input DVE op needs both
        # inputs on the same base partition, so: g at base 0 (own tile),
        # c/prod/tmp at base 32, tanh(c) at base 64 (next to o).
        c_state = pers.tile([64, 8 * 1024], F16, tag="cstate")
        prod = pers.tile([64, 8 * 1024], F16, tag="prod")
        tmp = pers.tile([64, 8 * 1024], F16, tag="tmp")
        tanhc = pers.tile([96, 8 * 1024], F16, tag="tanhc")
        hfin = pers.tile([32, 8 * 1024], F32, tag="hfin")
        zscr = pers.tile([CH, 1024], BF, tag="zscr")

        # ---- init ----
        nc.vector.memset(hstack[0][:, :], 0.0)
        nc.gpsimd.memset(hstack[1][:, :], 0.0)
        nc.vector.memset(c_state[32:64, :], 0.0)
        nc.vector.memset(zscr[:, :], 0.0)
        for i in range(2):
            nc.sync.dma_start(out=hstack[i][123:124, :], in_=ones_d[:, :])
        for i in range(2):
            nc.sync.dma_start(out=agin[i][2], in_=zscr[:, :])
        for _d in range(9):
            nc.sync.dma_start(out=wh_sb[:, 128 * _d:128 * (_d + 1)],
                              in_=whl_d[_d])
        nc.sync.dma_start(out=hstack[0][96:123, :], in_=xim_d[0])

        r_lo = nc.alloc_register(mybir.EngineType.Pool, "r_lo")
        r_hi = nc.alloc_register(mybir.EngineType.Pool, "r_hi")
        nc.reg_load(r_lo, hoff_d[0:1, 0:1])
        nc.reg_load(r_hi, hoff_d[0:1, 1:2])
        rv_lo = nc.snap(r_lo, min_val=0, max_val=11)
        rv_hi = nc.snap(r_hi, min_val=0, max_val=11)

        hsv = [h[:, :].rearrange("p (z y x) -> p z y x", z=SLAB, y=PLW, x=PLW)
               for h in hstack]

        def emit_plane(t, curv, gates, p):
            """9-delta matmul accumulation + gate activations for plane p."""
            ps = psum.tile([128, 1024], F32, tag="ps", name="ps")
            if t == 0:
                for h in range(2):
                    nc.tensor.matmul(ps[:, 512 * h:512 * (h + 1)],
                                     lhsT=wh_sb[:, 0:128],
                                     rhs=curv[:, p, 16 * h:16 * h + 16, 0:32],
                                     start=True, stop=True)
            else:
                for di, (dy, dx) in enumerate(DELTAS):
                    for h in range(2):
                        nc.tensor.matmul(
                            ps[:, 512 * h:512 * (h + 1)],
                            lhsT=wh_sb[:, 128 * di:128 * (di + 1)],
                            rhs=curv[:, p, 16 * h + dy:16 * h + dy + 16,
                                     dx:dx + 32],
                            start=(di == 0), stop=(di == 8))
            s = SPAN[p]
            nc.scalar.activation(gates[0:96, s:s + 1024], ps[0:96, :],
                                 mybir.ActivationFunctionType.Sigmoid)
            nc.scalar.activation(g_t[0:32, s:s + 1024], ps[96:128, :],
                                 mybir.ActivationFunctionType.Tanh)

        def emit_group(t, gates, g_t, nxtv, planes, s0, s1):
            """LSTM elementwise update for gate span [s0:s1] (planes list)."""
            i_sl = gates[0:32, s0:s1]
            f_sl = gates[32:64, s0:s1]
            o_sl = gates[64:96, s0:s1]
            c_sl = c_state[32:64, s0:s1]
            nc.vector.tensor_mul(prod[32:64, s0:s1], i_sl, g_t[0:32, s0:s1])
            nc.gpsimd.tensor_mul(tmp[32:64, s0:s1], f_sl, c_sl)
            nc.vector.tensor_add(c_sl, tmp[32:64, s0:s1], prod[32:64, s0:s1])
            nc.scalar.activation(tanhc[64:96, s0:s1], c_sl,
                                 mybir.ActivationFunctionType.Tanh)
            last = t == nsteps - 1
            for pl, a, b in planes:
                o_ap = o_sl[:, a - s0:b - s0].rearrange(
                    "p (z y x) -> p z y x", z=(b - a) // 1024, y=32, x=32)
                t_ap = tanhc[64:96, a:b].rearrange(
                    "p (z y x) -> p z y x", z=(b - a) // 1024, y=32, x=32)
                if last:
                    nc.vector.tensor_mul(
                        hfin[:, a:b].rearrange("p (z y x) -> p z y x",
                                               z=(b - a) // 1024, y=32, x=32),
                        o_ap, t_ap)
                else:
                    npl = (b - a) // 1024
                    nc.vector.tensor_mul(
                        nxtv[32:64, pl:pl + npl, 1:33, 1:33], o_ap, t_ap)

        T_ = nsteps
        for t in range(T_):
            cur, nxt = hstack[t % 2], hstack[(t + 1) % 2]
            curv, nxtv = hsv[t % 2], hsv[(t + 1) % 2]
            last = t == T_ - 1
            gates = work.tile([96, 8 * 1024], BF, tag="gates", name="gates")
            g_t = work.tile([32, 8 * 1024], BF, tag="g_t", name="g_t")
            if not last:
                nc.sync.dma_start(out=nxt[96:123, :], in_=xim_d[t + 1])

            # boundary planes first; their h feeds this step's collective
            emit_plane(t, curv, gates, 0)
            emit_plane(t, curv, gates, 7)
            emit_group(t, gates, g_t, nxtv, [(0, 0, 1024), (7, 1024, 2048)],
                       0, 2048)
            if not last:
                ag_i, ag_o = agin[t % 2], agout[t % 2]
                nc.sync.dma_start(
                    out=ag_i[0].rearrange("c (y x) -> c y x", y=32, x=32),
                    in_=nxtv[32:64, 0, 1:33, 1:33])
                nc.sync.dma_start(
                    out=ag_i[1].rearrange("c (y x) -> c y x", y=32, x=32),
                    in_=nxtv[32:64, 7, 1:33, 1:33])
                nc.gpsimd.collective_compute(
                    "AllGather", mybir.AluOpType.bypass, replica_groups=RG,
                    ins=[ag_i[:, :, :]], outs=[ag_o[:, :, :]])
                # g2 slot 6 <- h7 available right after the boundary group
                nc.sync.dma_start(out=nxtv[64:96, 6:7, :, :],
                                  in_=nxtv[32:64, 7:8, :, :])

            # interior planes in pairs (chunked elementwise + dz copies so
            # next-step matmuls wait only on per-plane producers)
            for p0 in (1, 3, 5):
                emit_plane(t, curv, gates, p0)
                emit_plane(t, curv, gates, p0 + 1)
                s0 = SPAN[p0]
                emit_group(t, gates, g_t, nxtv, [(p0, s0, s0 + 2048)],
                           s0, s0 + 2048)
                if not last:
                    # g0 slot p <- h[p-1]; g2 slot p <- h[p+1]
                    nc.sync.dma_start(out=nxtv[0:32, p0:p0 + 2, :, :],
                                      in_=nxtv[32:64, p0 - 1:p0 + 1, :, :])
                    nc.sync.dma_start(out=nxtv[64:96, p0 - 1:p0 + 1, :, :],
                                      in_=nxtv[32:64, p0:p0 + 2, :, :])

            if not last:
                # leftover replica slots: g0 slot 7 <- h6
                nc.sync.dma_start(out=nxtv[0:32, 7:8, :, :],
                                  in_=nxtv[32:64, 6:7, :, :])
                halo_lo = ag_o[bass.ds(rv_lo, 1)].squeeze(0).rearrange(
                    "c (y x) -> c y x", y=32, x=32)
                halo_hi = ag_o[bass.ds(rv_hi, 1)].squeeze(0).rearrange(
                    "c (y x) -> c y x", y=32, x=32)
                nc.gpsimd.dma_start(out=nxtv[0:32, 0, 1:33, 1:33], in_=halo_lo)
                nc.gpsimd.dma_start(out=nxtv[64:96, 7, 1:33, 1:33], in_=halo_hi)
            else:
                for pl in range(SLAB):
                    s = SPAN[pl]
                    nc.sync.dma_start(
                        out=hout_d[:, pl, :, :],
                        in_=hfin[:, s:s + 1024].rearrange(
                            "c (y x) -> c y x", y=32, x=32))

    nc.finalize()
    _prog_cache[key] = nc
    return nc


def _host_inputs(input_batch, Wx, Wh, b):
    input_batch = np.asarray(input_batch, dtype=np.float32)
    Wx = np.asarray(Wx, dtype=np.float32)
    Wh = np.asarray(Wh, dtype=np.float32)
    b = np.asarray(b, dtype=np.float32)

    xp = np.zeros((2, T, 66, 66, 66), np.float32)
    xp[:, :, 1:65, 1:65, 1:65] = input_batch[:, :, 0]

    whl = np.zeros((9, 128, 128), np.float32)
    for di, (dy, dx) in enumerate(DELTAS):
        for g in range(3):
            whl[di, 32 * g:32 * g + 32, :] = Wh[:, :, g, dy, dx].T
    whl[0, 96:123, :] = Wx[:, 0].reshape(128, 27).T
    whl[0, 123, :] = b
    whl = whl.astype(NPBF)

    in_maps = []
    for c in range(8):
        bidx, k = divmod(c, 4)
        z0 = 8 * k
        xim = np.zeros((T, 27, SLAB, PLW, PLW), np.float32)
        for tz in range(3):
            for ty in range(3):
                for tx in range(3):
                    tap = tz * 9 + ty * 3 + tx
                    xim[:, tap, :, 0:32, 0:32] = xp[
                        bidx, :, 2 * z0 + tz:2 * z0 + tz + 16:2,
                        ty:ty + 64:2, tx:tx + 64:2]
        lo_slot = 3 * k + 2 if k == 0 else 3 * (k - 1) + 1
        hi_slot = 3 * k + 2 if k == 3 else 3 * (k + 1)
        in_maps.append({
            "xim": xim.reshape(T, 27, HS_FREE).astype(NPBF),
            "whl": whl,
            "ones": np.ones((1, HS_FREE), NPBF),
            "hoff": np.array([[lo_slot, hi_slot]], np.int32),
        })
    return in_maps


def run_cores(in_maps, nsteps=T, **kwargs):
    nc = _build_program(nsteps)
    return run_bass_kernel_spmd(nc, in_maps, list(range(8)), **kwargs)


def kernel(input_batch, Wx, Wh, b):
    in_maps = _host_inputs(input_batch, Wx, Wh, b)
    res = run_cores(in_maps)
    out = np.zeros((2, CH, 32, 32, 32), np.float32)
    for c in range(8):
        bidx, k = divmod(c, 4)
        out[bidx, :, 8 * k:8 * k + 8] = res.results[c]["hout"]
    return out


# revision 10
# speedup vs baseline: 1.3907x; 1.0465x over previous
"""ConvLSTM3D encoder kernel for 8 trn2 NeuronCores (v2, bf16).

Sharding: core c in [0,8) handles batch b = c//4, z-slab k = c%4 (8 output
planes z in [8k, 8k+8)).  The sequential T=10 loop runs on-device; per-step
halo exchange (1 plane each side of the slab) is an AllGather over the 4
cores of each batch group (bf16 payload, double-buffered DRAM).

Conv mapping: gates = Wx (x) x_t (stride 2) + Wh (x) h + b is one K=128
bf16 matmul accumulation stream per output plane (N=1024 = full 32x32):
  partitions  0..95  : three z-shifted copies of h (dz = 0,1,2)
  partitions 96..122 : host-precomputed im2col taps of x_t (27 taps)
  partition  123     : ones (bias row, memset once)
For each (dy,dx) in 3x3, one matmul with an AP offset of (dy,dx) into the
padded (34x34) plane layout contracts channels x dz at once; the x-conv and
bias ride in the delta=(0,0) matmul only (their lhsT rows are zero in the
other eight).

Elementwise LSTM math runs on [32, span] slices straight out of the gates
tile (i/f/o/g live on partition quadrants 0:32/32:64/64:96/96:128 - the DVE
crossbar allows different quadrant bases per operand at nch=32), cell state
is fp16 for the 2-byte DVE fast mode.  h is written once (strided, bf16)
into the dz=1 partition group of the next h-stack; the dz=0/2 groups are
produced by two large contiguous SBUF->SBUF DMAs with a +-1 plane offset.
Boundary planes (0,7) are computed first each step so the halo collective
overlaps the interior-plane compute.
"""

import os
import sys
from contextlib import ExitStack

import numpy as np
import ml_dtypes

for _p in ("/opt/trn_rl_repo", "/root/.axon_site/_ro/trn_rl_repo"):
    if os.path.isdir(_p) and _p not in sys.path:
        sys.path.insert(0, _p)

import concourse.bass as bass
import concourse.bacc as bacc
import concourse.mybir as mybir
from concourse import tile
from concourse.bass_utils import run_bass_kernel_spmd

F32 = mybir.dt.float32
F16 = mybir.dt.float16
BF = mybir.dt.bfloat16
I32 = mybir.dt.int32

T = 10
CH = 32          # hidden channels
SLAB = 8         # output planes per core
PLW = 34         # padded plane width
PL = PLW * PLW   # padded plane elements (1156)
HS_FREE = SLAB * PL  # h-stack free size per partition (9248)
DELTAS = [(dy, dx) for dy in range(3) for dx in range(3)]
# plane processing order: boundary planes first (their h feeds the
# collective), interior after (overlaps the collective in flight)
PO = [0, 7, 1, 2, 3, 4, 5, 6]
SPAN = {p: i * 1024 for i, p in enumerate(PO)}
RG = [[0, 1, 2, 3], [4, 5, 6, 7]]
NPBF = ml_dtypes.bfloat16

_prog_cache = {}


def _build_program(nsteps=T):
    key = nsteps
    if key in _prog_cache:
        return _prog_cache[key]

    nc = bacc.Bacc(num_devices=8)

    xim_d = nc.dram_tensor("xim", [T, 27, HS_FREE], BF, kind="ExternalInput")
    whl_d = nc.dram_tensor("whl", [9, 128, 128], BF, kind="ExternalInput")
    hoff_d = nc.dram_tensor("hoff", [1, 2], I32, kind="ExternalInput")
    ones_d = nc.dram_tensor("ones", [1, HS_FREE], BF, kind="ExternalInput")
    hout_d = nc.dram_tensor("hout", [CH, SLAB, 32, 32], F32, kind="ExternalOutput")
    agin = [nc.dram_tensor(f"agin{i}", [3, CH, 1024], BF) for i in range(2)]
    agout = [nc.dram_tensor(f"agout{i}", [12, CH, 1024], BF)
             for i in range(2)]

    with ExitStack() as ctx:
        tc = ctx.enter_context(tile.TileContext(nc))
        pers = ctx.enter_context(tc.tile_pool(name="pers", bufs=1))
        psum = ctx.enter_context(tc.tile_pool(name="psum", bufs=4, space="PSUM"))
        work = ctx.enter_context(tc.tile_pool(name="work", bufs=2))

        hstack = [
            pers.tile([128, HS_FREE], BF, tag="hstackA", name="hstackA"),
            pers.tile([128, HS_FREE], BF, tag="hstackB", name="hstackB"),
        ]
        wh_sb = pers.tile([128, 9 * 128], BF, tag="wh")
        # elementwise operand placement: every two-input DVE op needs both
        # inputs on the same base partition, so: g at base 0 (own tile),
        # c/prod/tmp at base 32, tanh(c) at base 64 (next to o).
        c_state = pers.tile([64, 8 * 1024], F16, tag="cstate")
        prod = pers.tile([64, 8 * 1024], F16, tag="prod")
        tmp = pers.tile([64, 8 * 1024], F16, tag="tmp")
        tanhc = pers.tile([96, 8 * 1024], F16, tag="tanhc")
        hfin = pers.tile([32, 8 * 1024], F32, tag="hfin")
        zscr = pers.tile([CH, 1024], BF, tag="zscr")

        # ---- init ----
        nc.vector.memset(hstack[0][:, :], 0.0)
        nc.gpsimd.memset(hstack[1][:, :], 0.0)
        nc.vector.memset(c_state[32:64, :], 0.0)
        nc.vector.memset(zscr[:, :], 0.0)
        for i in range(2):
            nc.sync.dma_start(out=hstack[i][123:124, :], in_=ones_d[:, :])
        for i in range(2):
            nc.sync.dma_start(out=agin[i][2], in_=zscr[:, :])
        for _d in range(9):
            nc.sync.dma_start(out=wh_sb[:, 128 * _d:128 * (_d + 1)],
                              in_=whl_d[_d])
        nc.sync.dma_start(out=hstack[0][96:123, :], in_=xim_d[0])

        r_lo = nc.alloc_register(mybir.EngineType.Pool, "r_lo")
        r_hi = nc.alloc_register(mybir.EngineType.Pool, "r_hi")
        nc.reg_load(r_lo, hoff_d[0:1, 0:1])
        nc.reg_load(r_hi, hoff_d[0:1, 1:2])
        rv_lo = nc.snap(r_lo, min_val=0, max_val=11)
        rv_hi = nc.snap(r_hi, min_val=0, max_val=11)

        hsv = [h[:, :].rearrange("p (z y x) -> p z y x", z=SLAB, y=PLW, x=PLW)
               for h in hstack]

        def emit_plane(t, curv, gates, p):
            """9-delta matmul accumulation + gate activations for plane p."""
            ps = psum.tile([128, 1024], F32, tag="ps", name="ps")
            if t == 0:
                for h in range(2):
                    nc.tensor.matmul(ps[:, 512 * h:512 * (h + 1)],
                                     lhsT=wh_sb[:, 0:128],
                                     rhs=curv[:, p, 16 * h:16 * h + 16, 0:32],
                                     start=True, stop=True)
            else:
                for di, (dy, dx) in enumerate(DELTAS):
                    for h in range(2):
                        nc.tensor.matmul(
                            ps[:, 512 * h:512 * (h + 1)],
                            lhsT=wh_sb[:, 128 * di:128 * (di + 1)],
                            rhs=curv[:, p, 16 * h + dy:16 * h + dy + 16,
                                     dx:dx + 32],
                            start=(di == 0), stop=(di == 8))
            s = SPAN[p]
            nc.scalar.activation(gates[0:96, s:s + 1024], ps[0:96, :],
                                 mybir.ActivationFunctionType.Sigmoid)
            nc.scalar.activation(g_t[0:32, s:s + 1024], ps[96:128, :],
                                 mybir.ActivationFunctionType.Tanh)

        def emit_group(t, gates, g_t, nxtv, planes, s0, s1):
            """LSTM elementwise update for gate span [s0:s1] (planes list)."""
            i_sl = gates[0:32, s0:s1]
            f_sl = gates[32:64, s0:s1]
            o_sl = gates[64:96, s0:s1]
            c_sl = c_state[32:64, s0:s1]
            nc.vector.tensor_mul(prod[32:64, s0:s1], i_sl, g_t[0:32, s0:s1])
            nc.vector.tensor_mul(tmp[32:64, s0:s1], f_sl, c_sl)
            nc.vector.tensor_add(c_sl, tmp[32:64, s0:s1], prod[32:64, s0:s1])
            nc.scalar.activation(tanhc[64:96, s0:s1], c_sl,
                                 mybir.ActivationFunctionType.Tanh)
            last = t == nsteps - 1
            for pl, a, b in planes:
                o_ap = o_sl[:, a - s0:b - s0].rearrange(
                    "p (z y x) -> p z y x", z=(b - a) // 1024, y=32, x=32)
                t_ap = tanhc[64:96, a:b].rearrange(
                    "p (z y x) -> p z y x", z=(b - a) // 1024, y=32, x=32)
                if last:
                    nc.vector.tensor_mul(
                        hfin[:, a:b].rearrange("p (z y x) -> p z y x",
                                               z=(b - a) // 1024, y=32, x=32),
                        o_ap, t_ap)
                else:
                    npl = (b - a) // 1024
                    nc.vector.tensor_mul(
                        nxtv[32:64, pl:pl + npl, 1:33, 1:33], o_ap, t_ap)

        T_ = nsteps
        for t in range(T_):
            cur, nxt = hstack[t % 2], hstack[(t + 1) % 2]
            curv, nxtv = hsv[t % 2], hsv[(t + 1) % 2]
            last = t == T_ - 1
            gates = work.tile([96, 8 * 1024], BF, tag="gates", name="gates")
            g_t = work.tile([32, 8 * 1024], BF, tag="g_t", name="g_t")
            if not last:
                nc.sync.dma_start(out=nxt[96:123, :], in_=xim_d[t + 1])

            # boundary planes first; their h feeds this step's collective
            emit_plane(t, curv, gates, 0)
            emit_plane(t, curv, gates, 7)
            emit_group(t, gates, g_t, nxtv, [(0, 0, 1024), (7, 1024, 2048)],
                       0, 2048)
            if not last:
                ag_i, ag_o = agin[t % 2], agout[t % 2]
                nc.sync.dma_start(
                    out=ag_i[0].rearrange("c (y x) -> c y x", y=32, x=32),
                    in_=nxtv[32:64, 0, 1:33, 1:33])
                nc.sync.dma_start(
                    out=ag_i[1].rearrange("c (y x) -> c y x", y=32, x=32),
                    in_=nxtv[32:64, 7, 1:33, 1:33])
                nc.gpsimd.collective_compute(
                    "AllGather", mybir.AluOpType.bypass, replica_groups=RG,
                    ins=[ag_i[:, :, :]], outs=[ag_o[:, :, :]])
                # g2 slot 6 <- h7 available right after the boundary group
                nc.sync.dma_start(out=nxtv[64:96, 6:7, :, :],
                                  in_=nxtv[32:64, 7:8, :, :])

            # interior planes in pairs (chunked elementwise + dz copies so
            # next-step matmuls wait only on per-plane producers)
            for p0 in (1, 3, 5):
                emit_plane(t, curv, gates, p0)
                emit_plane(t, curv, gates, p0 + 1)
                s0 = SPAN[p0]
                emit_group(t, gates, g_t, nxtv, [(p0, s0, s0 + 2048)],
                           s0, s0 + 2048)
                if not last:
                    # g0 slot p <- h[p-1]; g2 slot p <- h[p+1]
                    nc.sync.dma_start(out=nxtv[0:32, p0:p0 + 2, :, :],
                                      in_=nxtv[32:64, p0 - 1:p0 + 1, :, :])
                    nc.sync.dma_start(out=nxtv[64:96, p0 - 1:p0 + 1, :, :],
                                      in_=nxtv[32:64, p0:p0 + 2, :, :])

            if not last:
                # leftover replica slots: g0 slot 7 <- h6
                nc.sync.dma_start(out=nxtv[0:32, 7:8, :, :],
                                  in_=nxtv[32:64, 6:7, :, :])
                halo_lo = ag_o[bass.ds(rv_lo, 1)].squeeze(0).rearrange(
                    "c (y x) -> c y x", y=32, x=32)
                halo_hi = ag_o[bass.ds(rv_hi, 1)].squeeze(0).rearrange(
                    "c (y x) -> c y x", y=32, x=32)
                nc.gpsimd.dma_start(out=nxtv[0:32, 0, 1:33, 1:33], in_=halo_lo)
                nc.gpsimd.dma_start(out=nxtv[64:96, 7, 1:33, 1:33], in_=halo_hi)
            else:
                for pl in range(SLAB):
                    s = SPAN[pl]
                    nc.sync.dma_start(
                        out=hout_d[:, pl, :, :],
                        in_=hfin[:, s:s + 1024].rearrange(
                            "c (y x) -> c y x", y=32, x=32))

    nc.finalize()
    _prog_cache[key] = nc
    return nc


def _host_inputs(input_batch, Wx, Wh, b):
    input_batch = np.asarray(input_batch, dtype=np.float32)
    Wx = np.asarray(Wx, dtype=np.float32)
    Wh = np.asarray(Wh, dtype=np.float32)
    b = np.asarray(b, dtype=np.float32)

    xp = np.zeros((2, T, 66, 66, 66), np.float32)
    xp[:, :, 1:65, 1:65, 1:65] = input_batch[:, :, 0]

    whl = np.zeros((9, 128, 128), np.float32)
    for di, (dy, dx) in enumerate(DELTAS):
        for g in range(3):
            whl[di, 32 * g:32 * g + 32, :] = Wh[:, :, g, dy, dx].T
    whl[0, 96:123, :] = Wx[:, 0].reshape(128, 27).T
    whl[0, 123, :] = b
    whl = whl.astype(NPBF)

    in_maps = []
    for c in range(8):
        bidx, k = divmod(c, 4)
        z0 = 8 * k
        xim = np.zeros((T, 27, SLAB, PLW, PLW), np.float32)
        for tz in range(3):
            for ty in range(3):
                for tx in range(3):
                    tap = tz * 9 + ty * 3 + tx
                    xim[:, tap, :, 0:32, 0:32] = xp[
                        bidx, :, 2 * z0 + tz:2 * z0 + tz + 16:2,
                        ty:ty + 64:2, tx:tx + 64:2]
        lo_slot = 3 * k + 2 if k == 0 else 3 * (k - 1) + 1
        hi_slot = 3 * k + 2 if k == 3 else 3 * (k + 1)
        in_maps.append({
            "xim": xim.reshape(T, 27, HS_FREE).astype(NPBF),
            "whl": whl,
            "ones": np.ones((1, HS_FREE), NPBF),
            "hoff": np.array([[lo_slot, hi_slot]], np.int32),
        })
    return in_maps


def run_cores(in_maps, nsteps=T, **kwargs):
    nc = _build_program(nsteps)
    return run_bass_kernel_spmd(nc, in_maps, list(range(8)), **kwargs)


def kernel(input_batch, Wx, Wh, b):
    in_maps = _host_inputs(input_batch, Wx, Wh, b)
    res = run_cores(in_maps)
    out = np.zeros((2, CH, 32, 32, 32), np.float32)
    for c in range(8):
        bidx, k = divmod(c, 4)
        out[bidx, :, 8 * k:8 * k + 8] = res.results[c]["hout"]
    return out
